# revision 8
# baseline (speedup 1.0000x reference)
"""Trainium2 Bass kernel for a per-channel linear recurrence (cumulative
mul-sum): y[b, t, c] = d[c] * y[b, t-1, c] + x[b, t, c], with y starting
at 0 (so y[b, 0] = x[b, 0]).

Full inputs x:[8, 4096, 1024] f32, d:[1024] f32 -> y:[8, 4096, 1024] f32.
Data-parallel over the batch dim: core b computes batch b (zero
communication).

Per-core pipeline (software-pipelined across 512-seq chunks):
  1. contiguous 512 KiB DMA loads bring 4 seq-blocks [128, 1024] per chunk
  2. PE transposes each 128x128 block (identity matmul) into PSUM chunks
     [128 ch, 512 seq]
  3. VectorE tensor_tensor_scan (state = d*state + x) runs along the free
     (seq) axis straight out of PSUM; chunks chained via initial=prev[:, -1:]
  4. PE transposes the scan result back via PSUM, ScalarE copies
     PSUM -> SBUF natural-layout staging
  5. contiguous 512 KiB DMA stores per seq-block

Emission order keeps all of a chunk's input transposes ahead of any
out-transposes on the PE queue (out-transposes wait on scans), so PE
never stalls the next group's inputs behind a scan.
"""

import numpy as np

import concourse.bacc as bacc
import concourse.tile as tile
import concourse.mybir as mybir
from concourse import masks
from concourse import bass_utils

P = 128
BSZ = 8
SEQ = 4096
CDIM = 1024
CHUNK = 512

_NC_CACHE = {}


def _build_nc(finalize: bool = True, psin_bufs: int = 4, psout_bufs: int = 4,
              reps: int = 1):
    nc = bacc.Bacc("TRN2", target_bir_lowering=False, debug=False)
    x = nc.dram_tensor("x", [SEQ, CDIM], mybir.dt.float32, kind="ExternalInput")
    d = nc.dram_tensor("d", [CDIM], mybir.dt.float32, kind="ExternalInput")
    y = nc.dram_tensor("y", [SEQ, CDIM], mybir.dt.float32, kind="ExternalOutput")

    G = CDIM // P        # 8 channel groups
    BPC = CHUNK // P     # 4 seq blocks per chunk
    NCH = SEQ // CHUNK   # 8 chunks
    fp32 = mybir.dt.float32

    with tile.TileContext(nc) as tc:
        with (
            tc.tile_pool(name="singles", bufs=1) as singles,
            tc.tile_pool(name="xb_pool", bufs=3 * BPC) as xb_pool,
            tc.tile_pool(name="yt_pool", bufs=2 * G) as yt_pool,
            tc.tile_pool(name="ynat_pool", bufs=2) as ynat_pool,
            tc.tile_pool(name="psin_pool", bufs=psin_bufs, space="PSUM") as psin_pool,
            tc.tile_pool(name="psout_pool", bufs=psout_bufs, space="PSUM") as psout_pool,
        ):
            identity = singles.tile([P, P], fp32)
            masks.make_identity(nc, identity[:])
            dcol = singles.tile([P, G], fp32)
            nc.sync.dma_start(out=dcol[:, :], in_=d.ap().rearrange("(g p) -> p g", p=P))
            dbc = singles.tile([P, G * CHUNK], fp32)
            nc.vector.memset(dbc[:, :], 1.0)
            for g in range(G):
                nc.vector.tensor_scalar_mul(
                    dbc[:, g * CHUNK:(g + 1) * CHUNK],
                    dbc[:, g * CHUNK:(g + 1) * CHUNK],
                    dcol[:, g:g + 1],
                )

            def load_chunk(k):
                xb = []
                for jj in range(BPC):
                    j = k * BPC + jj
                    t = xb_pool.tile([P, CDIM], fp32, name="xb", tag="xb")
                    nc.sync.dma_start(out=t[:, :], in_=x[j * P:(j + 1) * P, :])
                    xb.append(t)
                return xb

            def body():
              prev_yt = [None] * G
              xb_cur = load_chunk(0)
              for k in range(NCH):
                ps_ins = []
                for g in range(G):
                    ps_in = psin_pool.tile([P, CHUNK], fp32, name="ps_in", tag="ps_in")
                    for jj in range(BPC):
                        nc.tensor.transpose(
                            ps_in[:, jj * P:(jj + 1) * P],
                            xb_cur[jj][:, g * P:(g + 1) * P],
                            identity[:],
                        )
                    ps_ins.append(ps_in)
                xb_next = load_chunk(k + 1) if k + 1 < NCH else None
                yts = []
                for g in range(G):
                    yt = yt_pool.tile([P, CHUNK], fp32, name="yt", tag="yt")
                    init = 0.0 if prev_yt[g] is None else prev_yt[g][:, CHUNK - 1:CHUNK]
                    nc.vector.tensor_tensor_scan(
                        out=yt[:, :],
                        data0=dbc[:, g * CHUNK:(g + 1) * CHUNK],
                        data1=ps_ins[g][:, :],
                        initial=init,
                        op0=mybir.AluOpType.mult,
                        op1=mybir.AluOpType.add,
                    )
                    prev_yt[g] = yt
                    yts.append(yt)
                ynat = ynat_pool.tile([P, BPC * CDIM], fp32, name="ynat", tag="ynat")
                for g in range(G):
                    ps_out = psout_pool.tile(
                        [P, CHUNK], fp32, name="ps_out", tag="ps_out"
                    )
                    for jj in range(BPC):
                        nc.tensor.transpose(
                            ps_out[:, jj * P:(jj + 1) * P],
                            yts[g][:, jj * P:(jj + 1) * P],
                            identity[:],
                        )
                    for jj in range(BPC):
                        nc.scalar.copy(
                            out=ynat[:, jj * CDIM + g * P: jj * CDIM + (g + 1) * P],
                            in_=ps_out[:, jj * P:(jj + 1) * P],
                        )
                for jj in range(BPC):
                    j = k * BPC + jj
                    nc.sync.dma_start(
                        out=y[j * P:(j + 1) * P, :],
                        in_=ynat[:, jj * CDIM:(jj + 1) * CDIM],
                    )
                xb_cur = xb_next

            if reps == 1:
                body()
            else:
                with tc.For_i(0, reps, 1):
                    body()

    if finalize:
        nc.finalize()
    return nc


def _get_nc():
    if "nc" not in _NC_CACHE:
        _NC_CACHE["nc"] = _build_nc()
    return _NC_CACHE["nc"]


def kernel(x: np.ndarray, d: np.ndarray, **run_kwargs) -> np.ndarray:
    assert x.shape == (BSZ, SEQ, CDIM), x.shape
    assert d.shape == (CDIM,), d.shape
    x = np.ascontiguousarray(x, dtype=np.float32)
    d = np.ascontiguousarray(d, dtype=np.float32)

    nc = _get_nc()
    in_maps = [{"x": x[b], "d": d} for b in range(BSZ)]
    res = bass_utils.run_bass_kernel_spmd(
        nc, in_maps, core_ids=list(range(BSZ)), **run_kwargs
    )
    out = np.stack([res.results[b]["y"] for b in range(BSZ)], axis=0)
    _NC_CACHE["last_results"] = res
    return out


# revision 10
# speedup vs baseline: 1.2301x; 1.2301x over previous
"""Trainium2 Bass kernel for a per-channel linear recurrence (cumulative
mul-sum): y[b, t, c] = d[c] * y[b, t-1, c] + x[b, t, c], with y starting
at 0 (so y[b, 0] = x[b, 0]).

Full inputs x:[8, 4096, 1024] f32, d:[1024] f32 -> y:[8, 4096, 1024] f32.
Data-parallel over the batch dim: core b computes batch b (zero
communication).

Per-core pipeline (software-pipelined across 512-seq chunks):
  1. contiguous 512 KiB DMA loads bring 4 seq-blocks [128, 1024] per chunk
  2. PE transposes each 128x128 block (identity matmul) into PSUM chunks
     [128 ch, 512 seq]
  3. VectorE tensor_tensor_scan (state = d*state + x) runs along the free
     (seq) axis straight out of PSUM; chunks chained via initial=prev[:, -1:]
  4. PE transposes the scan result back via PSUM; ScalarE scatters each
     group's four blocks into natural-layout SBUF staging with ONE
     strided copy (4x fewer ACT instructions than per-block copies)
  5. contiguous 512 KiB DMA stores per seq-block

Emission order keeps all of a chunk's input transposes ahead of any
out-transposes on the PE queue (out-transposes wait on scans), so PE
never stalls the next group's inputs behind a scan. Measured on HW via
in-NEFF For_i amplification: ~79 us/core (vs ~110 us for the naive
ordering; DMA probes show ~1.2 TB/s loads and 13 ns PE transposes, so
the remaining time is the DVE scan chain (~39 us floor) plus ACT copy
overlap).
"""

import numpy as np

import concourse.bacc as bacc
import concourse.tile as tile
import concourse.mybir as mybir
from concourse import masks
from concourse import bass_utils

P = 128
BSZ = 8
SEQ = 4096
CDIM = 1024
CHUNK = 512

_NC_CACHE = {}


def _build_nc(finalize: bool = True, psin_bufs: int = 4, psout_bufs: int = 4,
              reps: int = 1):
    nc = bacc.Bacc("TRN2", target_bir_lowering=False, debug=False)
    x = nc.dram_tensor("x", [SEQ, CDIM], mybir.dt.float32, kind="ExternalInput")
    d = nc.dram_tensor("d", [CDIM], mybir.dt.float32, kind="ExternalInput")
    y = nc.dram_tensor("y", [SEQ, CDIM], mybir.dt.float32, kind="ExternalOutput")

    G = CDIM // P        # 8 channel groups
    BPC = CHUNK // P     # 4 seq blocks per chunk
    NCH = SEQ // CHUNK   # 8 chunks
    fp32 = mybir.dt.float32

    with tile.TileContext(nc) as tc:
        with (
            tc.tile_pool(name="singles", bufs=1) as singles,
            tc.tile_pool(name="xb_pool", bufs=3 * BPC) as xb_pool,
            tc.tile_pool(name="yt_pool", bufs=2 * G) as yt_pool,
            tc.tile_pool(name="ynat_pool", bufs=2) as ynat_pool,
            tc.tile_pool(name="psin_pool", bufs=psin_bufs, space="PSUM") as psin_pool,
            tc.tile_pool(name="psout_pool", bufs=psout_bufs, space="PSUM") as psout_pool,
        ):
            identity = singles.tile([P, P], fp32)
            masks.make_identity(nc, identity[:])
            dcol = singles.tile([P, G], fp32)
            nc.sync.dma_start(out=dcol[:, :], in_=d.ap().rearrange("(g p) -> p g", p=P))
            dbc = singles.tile([P, G * CHUNK], fp32)
            nc.vector.memset(dbc[:, :], 1.0)
            for g in range(G):
                nc.vector.tensor_scalar_mul(
                    dbc[:, g * CHUNK:(g + 1) * CHUNK],
                    dbc[:, g * CHUNK:(g + 1) * CHUNK],
                    dcol[:, g:g + 1],
                )

            def load_chunk(k):
                xb = []
                for jj in range(BPC):
                    j = k * BPC + jj
                    t = xb_pool.tile([P, CDIM], fp32, name="xb", tag="xb")
                    nc.sync.dma_start(out=t[:, :], in_=x[j * P:(j + 1) * P, :])
                    xb.append(t)
                return xb

            def body():
              prev_yt = [None] * G
              xb_cur = load_chunk(0)
              for k in range(NCH):
                ps_ins = []
                for g in range(G):
                    ps_in = psin_pool.tile([P, CHUNK], fp32, name="ps_in", tag="ps_in")
                    for jj in range(BPC):
                        nc.tensor.transpose(
                            ps_in[:, jj * P:(jj + 1) * P],
                            xb_cur[jj][:, g * P:(g + 1) * P],
                            identity[:],
                        )
                    ps_ins.append(ps_in)
                xb_next = load_chunk(k + 1) if k + 1 < NCH else None
                yts = []
                for g in range(G):
                    yt = yt_pool.tile([P, CHUNK], fp32, name="yt", tag="yt")
                    init = 0.0 if prev_yt[g] is None else prev_yt[g][:, CHUNK - 1:CHUNK]
                    nc.vector.tensor_tensor_scan(
                        out=yt[:, :],
                        data0=dbc[:, g * CHUNK:(g + 1) * CHUNK],
                        data1=ps_ins[g][:, :],
                        initial=init,
                        op0=mybir.AluOpType.mult,
                        op1=mybir.AluOpType.add,
                    )
                    prev_yt[g] = yt
                    yts.append(yt)
                ynat = ynat_pool.tile([P, BPC * CDIM], fp32, name="ynat", tag="ynat")
                ynat_r = ynat[:, :].rearrange("p (j c) -> p j c", c=CDIM)
                for g in range(G):
                    ps_out = psout_pool.tile(
                        [P, CHUNK], fp32, name="ps_out", tag="ps_out"
                    )
                    for jj in range(BPC):
                        nc.tensor.transpose(
                            ps_out[:, jj * P:(jj + 1) * P],
                            yts[g][:, jj * P:(jj + 1) * P],
                            identity[:],
                        )
                    # one strided copy scatters all 4 blocks of this group
                    nc.scalar.copy(
                        out=ynat_r[:, :, g * P:(g + 1) * P],
                        in_=ps_out[:, :].rearrange("p (j c) -> p j c", c=P),
                    )
                for jj in range(BPC):
                    j = k * BPC + jj
                    nc.sync.dma_start(
                        out=y[j * P:(j + 1) * P, :],
                        in_=ynat[:, jj * CDIM:(jj + 1) * CDIM],
                    )
                xb_cur = xb_next

            if reps == 1:
                body()
            else:
                with tc.For_i(0, reps, 1):
                    body()

    if finalize:
        nc.finalize()
    return nc


def _get_nc():
    if "nc" not in _NC_CACHE:
        _NC_CACHE["nc"] = _build_nc()
    return _NC_CACHE["nc"]


def kernel(x: np.ndarray, d: np.ndarray, **run_kwargs) -> np.ndarray:
    assert x.shape == (BSZ, SEQ, CDIM), x.shape
    assert d.shape == (CDIM,), d.shape
    x = np.ascontiguousarray(x, dtype=np.float32)
    d = np.ascontiguousarray(d, dtype=np.float32)

    nc = _get_nc()
    in_maps = [{"x": x[b], "d": d} for b in range(BSZ)]
    res = bass_utils.run_bass_kernel_spmd(
        nc, in_maps, core_ids=list(range(BSZ)), **run_kwargs
    )
    out = np.stack([res.results[b]["y"] for b in range(BSZ)], axis=0)
    _NC_CACHE["last_results"] = res
    return out
